# revision 1
# baseline (speedup 1.0000x reference)
# Trainium2 Bass kernel for nn_Attention_65609920413963 (sparse block-masked
# attention), v2: per-batch exact shapes + uniform head-pair sharding.
#
# Math structure (same as v1, verified against reference numerics):
#   L_b = n1[b]*n2[b].  Rows >= L_b are fully masked -> exactly uniform
#   softmax -> host computes mean(V) @ proj_w.T + proj_b for them.  Rows
#   f < L_b see keys k < L_b only, with multiplicative bias exp(K[b,f,k])
#   folded into the post-exp multiply (ek), zeros marking masked keys.
#
# Sharding: every core runs the SAME program on head-pair j = core_id of ALL
# 4 batches (channels [128j, 128j+128) of Q/K/V).  Per-batch work is exact:
# queries F_b = L_b, keys padded to K_b = ceil(L_b/128) tiles.  This balances
# the 8 cores perfectly (identical work) and loads each weight slice once.
# Final projection partials (one per core, 128-channel contraction) are
# summed on the host in f32.
#
# Per-core pipeline per batch (PE dtypes: fp16 for x/Wq/Wk -> logits path,
# bf16 for V/P/proj path; PSUM accum is f32):
#   QT/KT [128ch, rows] = Wq/Wk.T @ x.T    (scale folded into Wq on host)
#   V     [keys, 128ch] = x @ Wv           -> vp per head with a ones column
#                                             (96/0) carrying softmax denoms
#   ST_h  [keys, rows]  = K_h @ Q_h.T      (even/odd heads on PE row halves)
#   PT_h  = exp(ST_h - 44) * ek            (ACT exp from PSUM, Pool multiply)
#   OT_h  [ch+denom, rows] accumulated over key tiles (denom col 64/0)
#   denom reciprocal -> broadcast to 128 partitions via a tiny matmul
#   Y     [rows, 1024] partial = (OT_h/denom) @ proj_w_slice  (bf16 out)
import os as _os

# The legacy tile scheduler emits an instruction order that faults the device
# on this program shape; the v2 ASAP scheduler is correct.
_os.environ.setdefault("TILE_SCHEDULER", "asap")

import numpy as np

B, N, C = 4, 1024, 1024
H, Dh = 16, 64
NCC = C // 128  # 8 contraction chunks

_CACHE = {}


def _shapes(Ls):
    F = [int(l) for l in Ls]                      # exact query rows
    K = [-(-f // 128) for f in F]                 # key tiles
    XW = [k * 128 for k in K]                     # padded key/x width
    R = [-(-f // 128) for f in F]                 # row tiles for proj
    xoff = np.cumsum([0] + F).tolist()            # offsets into xt packing (valid rows only)
    yoff = np.cumsum([0] + F).tolist()            # offsets into y/qt/otn
    ekoff = np.cumsum([0] + [K[b] * F[b] for b in range(len(F))]).tolist()
    koff = np.cumsum([0] + K).tolist()            # vp key-tile offsets
    return F, K, XW, R, xoff, yoff, ekoff, koff


def _chunks(total, cap=512):
    n = -(-total // cap)
    base = -(-total // n)
    out = []
    off = 0
    while off < total:
        w = min(base, total - off)
        out.append((off, w))
        off += w
    return out


def _build_program(key, reps=1):
    import os
    BISECT = int(os.environ.get("BISECT", "5"))
    # ASAP cannot schedule control-flow programs; the reps>1 timing build
    # falls back to the legacy scheduler (kernel() itself always uses reps=1).
    if reps > 1 or os.environ.get("FORCE_LEGACY"):
        os.environ.pop("TILE_SCHEDULER", None)
    else:
        os.environ["TILE_SCHEDULER"] = "asap"
    import concourse.bacc as bacc
    import concourse.bass as bass
    import concourse.mybir as mybir
    import concourse.tile as tile
    import contextlib

    Ls = list(key)
    F, K, XW, R, xoff, yoff, ekoff, koff = _shapes(Ls)
    NB = len(F)
    FT = yoff[-1]          # total query rows (== xt width, valid rows only)
    EKW = ekoff[-1]        # total ek width
    KT = koff[-1]          # total key tiles

    F32 = mybir.dt.float32
    F16 = mybir.dt.float16
    BF16 = mybir.dt.bfloat16

    nc = bacc.Bacc("TRN2", target_bir_lowering=False, debug=False)

    xt_d = nc.dram_tensor("xt", [128, NCC, FT], F16, kind="ExternalInput")
    wq_d = nc.dram_tensor("wq", [128, NCC, 128], F16, kind="ExternalInput")
    wk_d = nc.dram_tensor("wk", [128, NCC, 128], F16, kind="ExternalInput")
    wv_d = nc.dram_tensor("wv", [128, NCC, 128], F16, kind="ExternalInput")
    pw_d = nc.dram_tensor("pw", [128, C], BF16, kind="ExternalInput")
    ek_d = nc.dram_tensor("ek", [128, EKW], BF16, kind="ExternalInput")
    y_d = nc.dram_tensor("y", [FT, C], BF16, kind="ExternalOutput")

    def vkey(b, kt):
        return min(128, F[b] - kt * 128)   # valid keys in tile kt

    with tile.TileContext(nc) as tc:
        with (
            tc.For_i(0, reps, 1) if reps > 1 else contextlib.nullcontext(),
            tc.tile_pool(name="yspool", bufs=4) as yspool,
            tc.tile_pool(name="work", bufs=3) as work,
            tc.tile_pool(name="ptpool", bufs=4) as ptpool,
            tc.tile_pool(name="singles", bufs=1) as singles,
            tc.tile_pool(name="psA", bufs=2, space="PSUM") as psA,
            tc.tile_pool(name="psST", bufs=4, space="PSUM") as psST,
            tc.tile_pool(name="psOT", bufs=2, space="PSUM") as psOT,
        ):
            # ---- resident SBUF tensors (xt last: address-layout sensitive) --
            nbias_sb = singles.tile([128, 1], F32, tag="nbias")
            sel_sb = singles.tile([128, 128], BF16, tag="sel")
            qt_sb = singles.tile([128, FT], F16, tag="qt")
            kt_sb = singles.tile([128, FT], F16, tag="kt")
            otn_sb = singles.tile([128, FT], BF16, tag="otn")
            vp_sb = singles.tile([128, KT, 2, 128], BF16, tag="vp")
            ek_sb = singles.tile([128, EKW], BF16, tag="ek")
            wq_sb = singles.tile([128, NCC, 128], F16, tag="wq")
            wk_sb = singles.tile([128, NCC, 128], F16, tag="wk")
            wv_sb = singles.tile([128, NCC, 128], F16, tag="wv")
            pw_sb = singles.tile([128, C], BF16, tag="pw")
            xt_sb = singles.tile([128, NCC, FT], F16, tag="xt")

            nc.vector.memset(nbias_sb, -44.0)
            # broadcast selectors: row 64 spreads the even-head denom recip to
            # partitions 0:64, row 0 spreads the odd-head recip to 64:128
            # (matmul base partition must be one of 0/32/64)
            nc.vector.memset(sel_sb, 0.0)
            nc.vector.memset(sel_sb[64:65, 0:64], 1.0)
            nc.vector.memset(sel_sb[0:1, 64:128], 1.0)

            # ---- input DMAs (SP HWDGE queue) -------------------------------
            import json as _json
            _bo = os.environ.get("BORDER")
            if _bo:
                border = _json.loads(_bo)
            else:
                border = [0, 3, 2, 1] if NB == 4 else list(range(NB))
            b0 = border[0]
            xchunks = _chunks(F[b0])
            nc.sync.dma_start(out=wq_sb, in_=wq_d.ap())

            def xdma(b, qo, ql, eng=None):
                (eng or nc.sync).dma_start(
                    out=xt_sb[:, :, xoff[b] + qo : xoff[b] + qo + ql],
                    in_=xt_d.ap()[:, :, xoff[b] + qo : xoff[b] + qo + ql],
                )

            def ekdma(b, eng=None):
                (eng or nc.sync).dma_start(
                    out=ek_sb[:, ekoff[b] : ekoff[b + 1]],
                    in_=ek_d.ap()[:, ekoff[b] : ekoff[b + 1]],
                )

            xdma(b0, *xchunks[0])
            nc.sync.dma_start(out=wk_sb, in_=wk_d.ap())
            for qo, ql in xchunks[1:]:
                xdma(b0, qo, ql)
            nc.sync.dma_start(out=wv_sb, in_=wv_d.ap())
            ekdma(b0)
            for b in border[1:]:
                xdma(b, 0, F[b])
                ekdma(b)
            nc.sync.dma_start(out=pw_sb, in_=pw_d.ap())

            # ---- per-batch phases, emitted as fine-grained thunks ----------
            # Weave: while batch b's attention chain (ACT exp + DVE/Pool mult)
            # runs, PE executes the next batch's QKV matmuls and the previous
            # batch's projection, so no engine waits on a serial phase.
            def qkv_thunks(b):
                ts = []
                for w_sb, t_sb in ((wq_sb, qt_sb), (wk_sb, kt_sb)):
                    for qo, ql in _chunks(F[b]):
                        def t(w_sb=w_sb, t_sb=t_sb, qo=qo, ql=ql, b=b):
                            if BISECT < 1:
                                return
                            ps = psA.tile([128, 512], F32, tag="ps")
                            for cc in range(NCC):
                                nc.tensor.matmul(
                                    ps[:, 0:ql],
                                    w_sb[:, cc],
                                    xt_sb[:, cc, xoff[b] + qo : xoff[b] + qo + ql],
                                    start=(cc == 0),
                                    stop=(cc == NCC - 1),
                                )
                            nc.vector.tensor_copy(
                                out=t_sb[:, xoff[b] + qo : xoff[b] + qo + ql],
                                in_=ps[:, 0:ql],
                            )
                        ts.append(t)
                for kt in range(K[b]):
                    def t(kt=kt, b=b):
                        if BISECT < 1:
                            return
                        vk = vkey(b, kt)
                        ps = psA.tile([128, 512], F32, tag="ps")
                        for cc in range(NCC):
                            nc.tensor.matmul(
                                ps[0:vk, 0:128],
                                xt_sb[:, cc, xoff[b] + kt * 128 : xoff[b] + kt * 128 + vk],
                                wv_sb[:, cc],
                                start=(cc == 0),
                                stop=(cc == NCC - 1),
                            )
                        # even head: V at cols 0:64, ones at col 64 (its OT
                        # reads only cols 0:65).  odd head: cols 1:128 (cols
                        # 1:64 unread-but-finite filler), ones at col 0.
                        # DVE only: Pool/gpsimd cannot access PSUM.
                        kk = koff[b] + kt
                        nc.vector.tensor_copy(
                            out=vp_sb[0:vk, kk, 0, 0:64], in_=ps[0:vk, 0:64]
                        )
                        nc.vector.tensor_copy(
                            out=vp_sb[0:vk, kk, 1, 1:128], in_=ps[0:vk, 1:128]
                        )
                        nc.vector.memset(vp_sb[:, kk, 0, 64:65], 1.0)
                        nc.vector.memset(vp_sb[:, kk, 1, 0:1], 1.0)
                    ts.append(t)
                return ts

            def attn_thunks(b):
                ts = []
                for qo, ql in _chunks(F[b]):
                    ot_pair = []
                    st_q = []

                    def emit_st(kt, qo=qo, ql=ql, b=b):
                        vk = vkey(b, kt)
                        st_e = psST.tile([128, 512], F32, tag="st", name="st_e")
                        st_o = psST.tile([128, 512], F32, tag="st", name="st_o")
                        for st, lo, hi in ((st_e, 0, 64), (st_o, 64, 128)):
                            nc.tensor.matmul(
                                st[0:vk, 0:ql],
                                kt_sb[lo:hi, xoff[b] + kt * 128 : xoff[b] + kt * 128 + vk],
                                qt_sb[lo:hi, yoff[b] + qo : yoff[b] + qo + ql],
                                start=True,
                                stop=True,
                            )
                        st_q.append((kt, st_e, st_o))

                    def emit_tail(qo=qo, ql=ql, b=b, ot_pair=ot_pair):
                        kt, st_e, st_o = st_q.pop(0)
                        vk = vkey(b, kt)
                        if kt == 0:
                            ot_e = psOT.tile([128, 512], F32, tag="ot", name="ot_e")
                            ot_o = psOT.tile([128, 512], F32, tag="ot", name="ot_o")
                            ot_pair.extend([ot_e, ot_o])
                        ot_e, ot_o = ot_pair
                        e0 = ekoff[b] + kt * F[b] + qo
                        for par, st, ot in ((0, st_e, ot_e), (1, st_o, ot_o)):
                            et = work.tile([128, 512], BF16, tag="et")
                            nc.scalar.activation(
                                out=et[0:vk, 0:ql], in_=st[0:vk, 0:ql],
                                func=mybir.ActivationFunctionType.Exp,
                                bias=nbias_sb[0:vk, :],
                            )
                            pt = ptpool.tile([128, 512], BF16, tag="pt")
                            eng = nc.vector if par == 0 else nc.gpsimd
                            eng.tensor_mul(
                                pt[0:vk, 0:ql], et[0:vk, 0:ql],
                                ek_sb[0:vk, e0 : e0 + ql],
                            )
                            mw = 65 if par == 0 else 128
                            nc.tensor.matmul(
                                ot[0:mw, 0:ql],
                                vp_sb[0:vk, koff[b] + kt, par, 0:mw],
                                pt[0:vk, 0:ql],
                                start=(kt == 0),
                                stop=(kt == K[b] - 1),
                            )

                    # ST-lead pipeline: PE runs kt+1's ST matmuls while kt's
                    # exp/mult chain drains (psST=4 holds two kt of tiles)
                    for kt in range(K[b]):
                        def t(kt=kt, es=emit_st, etl=emit_tail, kb=K[b]):
                            es(kt)
                            if kt >= 1:
                                etl()
                            if kt == kb - 1:
                                etl()
                        ts.append(t)
                    if False:
                        def t(kt=kt, qo=qo, ql=ql, b=b, ot_pair=ot_pair):
                            if BISECT < 2:
                                return
                            vk = vkey(b, kt)
                            if kt == 0:
                                ot_e = psOT.tile([128, 512], F32, tag="ot", name="ot_e")
                                ot_o = psOT.tile([128, 512], F32, tag="ot", name="ot_o")
                                ot_pair.extend([ot_e, ot_o])
                            ot_e, ot_o = ot_pair
                            st_e = psST.tile([128, 512], F32, tag="st")
                            st_o = psST.tile([128, 512], F32, tag="st")
                            e0 = ekoff[b] + kt * F[b] + qo
                            for st, lo, hi in ((st_e, 0, 64), (st_o, 64, 128)):
                                nc.tensor.matmul(
                                    st[0:vk, 0:ql],
                                    kt_sb[lo:hi, xoff[b] + kt * 128 : xoff[b] + kt * 128 + vk],
                                    qt_sb[lo:hi, yoff[b] + qo : yoff[b] + qo + ql],
                                    start=True,
                                    stop=True,
                                )
                            if BISECT < 3:
                                return
                            for par, st, ot in ((0, st_e, ot_e), (1, st_o, ot_o)):
                                et = work.tile([128, 512], BF16, tag="et")
                                nc.scalar.activation(
                                    out=et[0:vk, 0:ql], in_=st[0:vk, 0:ql],
                                    func=mybir.ActivationFunctionType.Exp,
                                    bias=nbias_sb[0:vk, :],
                                )
                                # pure-SBUF multiplies, split across DVE and
                                # Pool so both heads' chains run in parallel;
                                # ek zeros also apply the key mask
                                pt = ptpool.tile([128, 512], BF16, tag="pt")
                                eng = nc.vector if par == 0 else nc.gpsimd
                                eng.tensor_mul(
                                    pt[0:vk, 0:ql], et[0:vk, 0:ql],
                                    ek_sb[0:vk, e0 : e0 + ql],
                                )
                                mw = 65 if par == 0 else 128
                                nc.tensor.matmul(
                                    ot[0:mw, 0:ql],
                                    vp_sb[0:vk, koff[b] + kt, par, 0:mw],
                                    pt[0:vk, 0:ql],
                                    start=(kt == 0),
                                    stop=(kt == K[b] - 1),
                                )
                        ts.append(t)

                    def t(qo=qo, ql=ql, b=b, ot_pair=ot_pair):
                        if BISECT < 4:
                            return
                        ot_e, ot_o = ot_pair
                        # 1-partition-contraction matmuls crash the device, so
                        # broadcast with 64-partition contractions instead:
                        # rcb is zeroed (sel rows are zero except 0/64, and
                        # 0 * uninit could be NaN), recips land on rows 64/0.
                        rcb = work.tile([128, 512], BF16, tag="rc")
                        nc.vector.memset(rcb[:, 0:ql], 0.0)
                        with nc.allow_low_precision(reason="bf16 denom recip"):
                            nc.vector.reciprocal(
                                out=rcb[64:65, 0:ql], in_=ot_e[64:65, 0:ql]
                            )
                            nc.vector.reciprocal(
                                out=rcb[0:1, 0:ql], in_=ot_o[0:1, 0:ql]
                            )
                        rb_ps = psA.tile([128, 512], F32, tag="ps", name="rb_ps")
                        nc.tensor.matmul(
                            rb_ps[:, 0:ql], sel_sb[64:128, :], rcb[64:128, 0:ql],
                            start=True, stop=False,
                        )
                        nc.tensor.matmul(
                            rb_ps[:, 0:ql], sel_sb[0:64, :], rcb[0:64, 0:ql],
                            start=False, stop=True,
                        )
                        rb_sb = work.tile([128, 512], BF16, tag="rb")
                        nc.vector.tensor_copy(out=rb_sb[:, 0:ql], in_=rb_ps[:, 0:ql])
                        qa = yoff[b] + qo
                        nc.vector.tensor_mul(
                            otn_sb[0:64, qa : qa + ql],
                            ot_e[0:64, 0:ql], rb_sb[0:64, 0:ql],
                        )
                        nc.vector.tensor_mul(
                            otn_sb[64:128, qa : qa + ql],
                            ot_o[64:128, 0:ql], rb_sb[64:128, 0:ql],
                        )
                    ts.append(t)
                return ts

            def proj_thunks(b, use_act=False):
                ts = []
                ys_box = []
                for rt in range(R[b]):
                    def t(rt=rt, b=b, ys_box=ys_box):
                        if BISECT < 5:
                            return
                        if rt == 0 and not os.environ.get("NOYS"):
                            ys_box.append(
                                yspool.tile([128, max(R), C], BF16, tag="ys", name="ys")
                            )
                        ys = ys_box[0] if ys_box else None
                        rows = min(128, F[b] - rt * 128)
                        if os.environ.get("FULLROWS"):
                            rows = 128
                        for oc in range([2, 1][bool(os.environ.get("PJOC0"))]):
                            r0 = 0 if os.environ.get("PJOFF0") else yoff[b] + rt * 128
                            rr = 128 if os.environ.get("PJOFF0") else rows
                            pj = psA.tile([128, 512], F32, tag="ps", name="pj")
                            nc.tensor.matmul(
                                pj[0:rr, :],
                                otn_sb[:, r0 : r0 + rr],
                                pw_sb[:, oc * 512 : (oc + 1) * 512],
                                start=True,
                                stop=True,
                            )
                            if os.environ.get("NOPROJCOPY"):
                                continue
                            # alternate ACT/DVE (retest under current pools)
                            on_act = (rt * 2 + oc) % 2 == 0
                            if os.environ.get("ACTONLY"):
                                on_act = True
                            if on_act:
                                nc.scalar.activation(
                                    out=ys[0:rows, rt, oc * 512 : (oc + 1) * 512],
                                    in_=pj[0:rows, :],
                                    func=mybir.ActivationFunctionType.Copy,
                                )
                            else:
                                nc.vector.tensor_copy(
                                    out=ys[0:rows, rt, oc * 512 : (oc + 1) * 512],
                                    in_=pj[0:rows, :],
                                )
                        # plain per-row-tile DMA: rearranged DRAM writes have
                        # divergent row-order semantics between CoreSim and HW
                        if os.environ.get("NOYDMA"):
                            return
                        r0 = yoff[b] + rt * 128
                        nc.sync.dma_start(
                            out=y_d.ap()[r0 : r0 + rows, :],
                            in_=ys[0:rows, rt, :],
                        )
                    ts.append(t)
                return ts

            # ---- weave -----------------------------------------------------
            if os.environ.get("PAIRWEAVE") and not os.environ.get("WEAVE"):
                # paired weave: each weave window spans exactly 2 batches
                # (the proven-safe envelope); projection always serial.
                i = 0
                while i < len(border):
                    a = border[i]
                    bnext = border[i + 1] if i + 1 < len(border) else None
                    if i == 0:
                        for t in qkv_thunks(a):
                            t()
                    nxt = qkv_thunks(bnext) if bnext is not None else []
                    for t in attn_thunks(a):
                        t()
                        if nxt:
                            nxt.pop(0)()
                    while nxt:
                        nxt.pop(0)()
                    for t in proj_thunks(a):
                        t()
                    if bnext is not None:
                        for t in attn_thunks(bnext):
                            t()
                        for t in proj_thunks(bnext):
                            t()
                        if i + 2 < len(border):
                            for t in qkv_thunks(border[i + 2]):
                                t()
                    i += 2
            elif os.environ.get("WEAVELITE") and not os.environ.get("WEAVE"):
                # weave-lite: interleave only the next batch's QKV matmuls
                # into the attention stream (PE fill); projection stays
                # serial per batch (full proj weaving faults the device).
                for i, b in enumerate(border):
                    if i == 0:
                        for t in qkv_thunks(b):
                            t()
                    nxt = qkv_thunks(border[i + 1]) if i + 1 < len(border) else []
                    for t in attn_thunks(b):
                        t()
                        if nxt:
                            nxt.pop(0)()
                    while nxt:
                        nxt.pop(0)()
                    for t in proj_thunks(b):
                        t()
            elif not os.environ.get("WEAVE"):
                for i, b in enumerate(border):
                    for t in qkv_thunks(b):
                        t()
                    for t in attn_thunks(b):
                        t()
                    for t in proj_thunks(b):
                        t()
            else:
                pending_proj = []
                for i, b in enumerate(border):
                    if i == 0:
                        for t in qkv_thunks(b):
                            t()
                    nxt = qkv_thunks(border[i + 1]) if i + 1 < len(border) else []
                    for t in attn_thunks(b):
                        t()
                        if nxt:
                            nxt.pop(0)()
                        if pending_proj:
                            pending_proj.pop(0)()
                    while nxt or pending_proj:
                        if nxt:
                            nxt.pop(0)()
                        if pending_proj:
                            pending_proj.pop(0)()
                    pending_proj = proj_thunks(b, use_act=(i >= len(border) - 2))
                while pending_proj:
                    pending_proj.pop(0)()

    nc.compile()
    return nc


def _pad_for(L):
    # retained name for test.py compatibility: returns the program cache key
    return tuple(int(l) for l in L)


def _prep_inputs(key, x, K, n1, n2, qkv_w, qkv_b, proj_w):
    import ml_dtypes

    Ls = list(key)
    F, Kt, XW, R, xoff, yoff, ekoff, koff = _shapes(Ls)
    FT, EKW = yoff[-1], ekoff[-1]
    scale = np.float32(Dh**-0.5)
    assert not np.any(qkv_b), "nonzero qkv_b not supported by this kernel"
    bf16 = ml_dtypes.bfloat16

    # xt: [128, 8, FT] fp16 (valid rows only, batches concatenated)
    X_all = np.empty((FT, C), dtype=np.float16)
    for b in range(B):
        X_all[xoff[b] : xoff[b] + F[b]] = x[b, : F[b]]
    xt = np.ascontiguousarray(X_all.T.reshape(NCC, 128, FT).transpose(1, 0, 2))

    # ek: [128, EKW] bf16 multiplicative exp(K), zeros on padded/masked keys
    ekp = np.zeros((128, EKW), dtype=bf16)
    for b in range(B):
        E = np.zeros((XW[b], F[b]), dtype=np.float32)
        E[: F[b], :] = np.exp(K[b, : F[b], : F[b]].astype(np.float32)).T
        ekp[:, ekoff[b] : ekoff[b + 1]] = (
            E.reshape(Kt[b], 128, F[b]).transpose(1, 0, 2).reshape(128, -1)
        )

    def wslice(w, j):
        # w rows [128j:128j+128] of [C, C]; -> [128 p(cc), 8 cc, 128 m]
        ws = np.ascontiguousarray(
            w[128 * j : 128 * (j + 1), :].T.reshape(NCC, 128, 128).transpose(1, 0, 2)
        )
        return ws

    in_maps = []
    for j in range(8):
        wq = wslice(qkv_w[0 * C : 1 * C] * scale, j).astype(np.float16)
        wk = wslice(qkv_w[1 * C : 2 * C], j).astype(np.float16)
        wv = wslice(qkv_w[2 * C : 3 * C], j).astype(np.float16)
        pw = np.ascontiguousarray(
            proj_w[:, 128 * j : 128 * (j + 1)].T
        ).astype(bf16)
        in_maps.append(
            {"xt": xt, "wq": wq, "wk": wk, "wv": wv, "pw": pw, "ek": ekp}
        )
    L = np.asarray(Ls, dtype=np.int32)
    return in_maps, L


def run_device(inputs, trace=False):
    """Compile (cached), run on 8 cores, return (BassKernelResults, L)."""
    from concourse import bass_utils

    x = np.asarray(inputs["x"], dtype=np.float32)
    K = np.asarray(inputs["K"], dtype=np.float32)
    n1 = np.asarray(inputs["n1"])
    n2 = np.asarray(inputs["n2"])
    L = (n1.astype(np.int64) * n2.astype(np.int64)).astype(np.int32)
    key = _pad_for(L)
    if ("nc", key) not in _CACHE:
        _CACHE[("nc", key)] = _build_program(key)
    nc = _CACHE[("nc", key)]

    in_maps, L = _prep_inputs(
        key, x, K, n1, n2,
        np.asarray(inputs["qkv_w"], dtype=np.float32),
        np.asarray(inputs["qkv_b"], dtype=np.float32),
        np.asarray(inputs["proj_w"], dtype=np.float32),
    )
    res = bass_utils.run_bass_kernel_spmd(
        nc, in_maps, core_ids=list(range(8)), trace=trace
    )
    return res, L


def kernel(**inputs):
    x = np.asarray(inputs["x"], dtype=np.float32)
    qkv_w = np.asarray(inputs["qkv_w"], dtype=np.float32)
    qkv_b = np.asarray(inputs["qkv_b"], dtype=np.float32)
    proj_w = np.asarray(inputs["proj_w"], dtype=np.float32)
    proj_b = np.asarray(inputs["proj_b"], dtype=np.float32)

    res, L = run_device(inputs)
    Fs, _, _, _, _, yoff, _, _ = _shapes(L)

    ysum = np.zeros((yoff[-1], C), dtype=np.float32)
    for r in res.results:
        ysum += np.asarray(r["y"], dtype=np.float32)
    ysum += proj_b

    out = np.empty((B, N, C), dtype=np.float32)
    for b in range(B):
        Lb = int(L[b])
        out[b, :Lb] = ysum[yoff[b] : yoff[b] + Lb]
        # fully-masked rows: exactly uniform softmax -> mean of V
        vbar = x[b].mean(axis=0) @ qkv_w[2 * C : 3 * C, :].T + qkv_b[2 * C : 3 * C]
        out[b, Lb:] = vbar @ proj_w.T + proj_b
    return out

